# revision 20
# baseline (speedup 1.0000x reference)
"""Trainium2 Bass kernel for a 2-layer GCN encoder + global mean pool.

Problem: x[100000,128] f32, edge_index[2,1600000] i64, batch[100000] i64
(sorted), W1/b1/W2/b2. Two GCNConv layers (symmetric deg^-1/2 norm, self
loops, relu) then mean-pool over 512 graphs -> [512, 128] f32.

Strategy (8 NeuronCores, data-parallel over graphs):
- Nodes partitioned by graph id into 8 contiguous shards (batch is sorted);
  each core owns the edges whose *destination* lands in its shard.
- Algebraic rewrite: aggregate-then-transform.  For each layer,
      h' = relu( (A_hat @ h) @ W + b ),   A_hat = D^-1/2 (A+I) D^-1/2
  so the sparse aggregation runs on raw 128-dim features and the dense
  matmul with W happens per 128-node window afterwards.  Self loops are
  NOT gathered: each window's own 128 rows are contiguous in the table,
  so they are streamed with a plain 2-descriptor DMA (from the per-core
  shard copy in layer 1, from the local h1 shard in layer 2 — no
  AllGather dependency) and folded in through a diagonal 1/deg block
  that opens each window's PSUM accumulation.
- Sparse aggregation per core: per 128-edge block, dma_gather fetches the
  128 source rows (bf16, 256B each) from the feature table; a host-built
  fp8-e4m3 selection matrix M_blk (one nonzero per edge row: its dst slot,
  valued with the GCN edge weight) is streamed from HBM as a plain
  contiguous DMA and scatters the gathered rows on the Tensor engine:
      PSUM[f, s] += G_blk.T @ M_blk
  accumulated over all blocks of a 128-node destination window.  The M
  matrices are identical for both layers (same graph), so one table in
  HBM serves both.  Padded gather lanes carry spread-out indices (a row-0
  default creates an HBM hotspot worth ~50% of gather time).
- dma_gather uses int16 indices, so the node table is addressed through
  <=32768-row "quarter" slices; edges are sorted by (window-group, quarter,
  window) so each gather call stays quarter-pure while the PSUM tiles of a
  4-window group stay resident across the group's quarter runs.
- The finished PSUM window (= Z^T) feeds the dense W matmul directly (its
  transposed layout is exactly the lhsT the Tensor engine wants); bias is
  added via a K=1 matmul with a ones row; relu on the Scalar engine.
- Between layers one 8-core AllGather shares the per-shard h1 table (bf16),
  written directly into local DRAM (gathering from Shared-space DRAM, or
  staging it through a copy, costs ~0.5 ms extra).
- Mean pool: host-built one-hot graph-slot matrices (streamed once) feed
  per-window matmuls into a single resident PSUM bank, then a
  reciprocal-count scale.

Host-side preprocessing is purely structural (index sorting, degree counts,
normalization coefficients and one-hot selection matrices derived from the
graph topology); all feature/weight compute runs on device.
"""

import math
import os

import numpy as np
import ml_dtypes

import concourse.bass as bass
import concourse.bacc as bacc
import concourse.mybir as mybir
import concourse.tile as tile
from concourse.bass_utils import run_bass_kernel_spmd

P = 128
C = 8               # cores
G = 512             # graphs
GPC = G // C        # graphs per core
F = 128             # feature dim (in = hid = out)
WG = 4              # windows per PSUM-resident group
QROWS = 32768       # rows per int16-addressable table slice

bf16 = mybir.dt.bfloat16
fp8 = mybir.dt.float8e4
f32 = mybir.dt.float32
i16 = mybir.dt.int16

BF = ml_dtypes.bfloat16
F8 = ml_dtypes.float8_e4m3

CALLBLK = int(os.environ.get("KERNEL_CALLBLK", "8"))


def _preprocess(x, edge_index, batch):
    """Structural preprocessing: shard nodes by graph, sort/pad edges by
    (window-group, src-quarter, dst-window), compute GCN norm weights and
    host-built selection matrices."""
    N = x.shape[0]
    src = np.asarray(edge_index[0], dtype=np.int64)
    dst = np.asarray(edge_index[1], dtype=np.int64)
    batch = np.asarray(batch, dtype=np.int64)

    node_start = np.searchsorted(batch, np.arange(C + 1) * GPC).astype(np.int64)
    nk = np.diff(node_start)
    NODE_PAD = int(math.ceil(nk.max() / P) * P)
    NW = NODE_PAD // P
    TOT = C * NODE_PAD
    assert TOT <= 4 * QROWS
    NG = (NW + WG - 1) // WG

    core_of = (batch // GPC).astype(np.int64)
    row = (np.arange(N) - node_start[core_of] + core_of * NODE_PAD).astype(np.int64)

    deg = np.bincount(dst, minlength=N).astype(np.float64) + 1.0
    dis = 1.0 / np.sqrt(deg)

    # real edges only; self loops are handled by streaming each window's
    # own (contiguous) rows through a diagonal 1/deg block instead of
    # spending 128 random-gather descriptors per window on them.
    esrc = src
    edst = dst
    ew = (dis[src] * dis[dst]).astype(np.float32)

    ecore = core_of[edst]
    eld = edst - node_start[ecore]
    ewin = eld >> 7
    eslot = (eld & 127).astype(np.int64)
    esrcrow = row[esrc]
    eq = (esrcrow // QROWS).astype(np.int64)      # source quarter
    eloc = (esrcrow % QROWS).astype(np.int64)     # quarter-local row
    egrp = ewin // WG

    NQ = 4
    # segment id in (core, group, quarter, window) order
    seg = ((ecore * NG + egrp) * NQ + eq) * NW + ewin
    NSEG = C * NG * NQ * NW
    counts = np.bincount(seg, minlength=NSEG)
    cnt4 = counts.reshape(C, NG, NQ, NW)
    # SPMD-common block counts per (group, quarter, window)
    BWS = np.ceil(cnt4.max(axis=0) / P).astype(np.int64)      # [NG, NQ, NW]
    for g in range(NG):
        mask = np.zeros(NW, bool)
        mask[g * WG:(g + 1) * WG] = True
        BWS[g, :, ~mask] = 0
    # every window needs >= 1 block so its PSUM tile is always written
    for w in range(NW):
        g = w // WG
        if BWS[g, :, w].sum() == 0:
            BWS[g, 0, w] = 1
    NBLK = int(BWS.sum())

    # block/segment offsets in (g, q, w) order
    seg_order = []          # (g, q, w, block_start, nblocks)
    seg_start = np.zeros((NG, NQ, NW), np.int64)
    acc = 0
    for g in range(NG):
        for q in range(NQ):
            for w in range(g * WG, min((g + 1) * WG, NW)):
                seg_start[g, q, w] = acc
                nb = int(BWS[g, q, w])
                if nb:
                    seg_order.append((g, q, w, acc, nb))
                acc += nb
    assert acc == NBLK

    # scatter edges into the padded per-core layout
    order = np.argsort(seg, kind="stable")
    seg_sorted = seg[order]
    grp_excl = np.concatenate([[0], np.cumsum(counts)[:-1]])
    pos = np.arange(order.size) - grp_excl[seg_sorted]
    es = order
    dest = (ecore[es] * (NBLK * P)
            + seg_start[egrp[es], eq[es], ewin[es]] * P + pos)

    # Padded lanes carry zero weight in M, so their index value is
    # correctness-neutral — spread them uniformly over the quarter instead
    # of defaulting to row 0 (duplicate descriptors to one HBM row create a
    # bank hotspot that slows the whole gather by ~50%).
    blk_q = np.zeros(NBLK, np.int64)
    for (g, q, w, b0, nb) in seg_order:
        blk_q[b0:b0 + nb] = q
    qrows_of_blk = np.where(blk_q < (TOT // QROWS), QROWS, TOT - (TOT // QROWS) * QROWS)
    if os.environ.get("KERNEL_NEG_PAD", "1") == "1":
        # pads sit at each segment's tail -> negative = skipped by ucode
        idx_arr = np.full(C * NBLK * P, -1, np.int16)
    else:
        prng = np.random.default_rng(12345)
        spread = (prng.integers(0, 1 << 30, size=(C, NBLK, P))
                  % qrows_of_blk[None, :, None])
        idx_arr = spread.reshape(-1).astype(np.int16)
    slot_arr = np.zeros(C * NBLK * P, np.int64)
    w_arr = np.zeros(C * NBLK * P, np.float32)
    idx_arr[dest] = eloc[es].astype(np.int16)
    slot_arr[dest] = eslot[es]
    w_arr[dest] = ew[es]

    # a block with zero real lanes on some core would make an all-negative
    # call; give its lane 0 a benign valid index (weight stays 0)
    has_real = np.zeros(C * NBLK * P, bool)
    has_real[dest] = True
    empty_blk = ~has_real.reshape(C, NBLK, P).any(axis=2)
    ia3 = idx_arr.reshape(C, NBLK, P)
    ia3[:, :, 0] = np.where(empty_blk, 0, ia3[:, :, 0])
    idx_arr = ia3.reshape(-1)

    if os.environ.get("KERNEL_SORT_LANES", "0") == "1":
        # sort lanes within each 128-lane block by gather index
        ia = idx_arr.reshape(C, NBLK, P)
        sa = slot_arr.reshape(C, NBLK, P)
        wa = w_arr.reshape(C, NBLK, P)
        o = np.argsort(ia, axis=2, kind="stable")
        idx_arr = np.take_along_axis(ia, o, 2).reshape(-1)
        slot_arr = np.take_along_axis(sa, o, 2).reshape(-1)
        w_arr = np.take_along_axis(wa, o, 2).reshape(-1)

    # per-core uploads
    # idx: wrapped [16, NBLK*8] (logical i at [i%16, i//16]), replicated to
    # 128 partitions (the gather ucode's per-Q7-core channel groups all read
    # the same wrap)
    idx_pc = np.ascontiguousarray(
        idx_arr.reshape(C, NBLK * P // 16, 16).transpose(0, 2, 1))
    idx_pc = np.ascontiguousarray(np.tile(idx_pc, (1, 8, 1)))

    # host-built selection matrices, transposed layout [128, NBLK*128]:
    # partition p holds, for each block b, the row M_b[p, :] (one nonzero
    # at the dst slot of the edge in lane p of block b).
    m_pc = np.zeros((C, P, NBLK * P), F8)
    sl = slot_arr.reshape(C, NBLK, P)
    wv = w_arr.reshape(C, NBLK, P)
    for c in range(C):
        m = np.zeros((NBLK, P, P), np.float32)
        np.put_along_axis(m, sl[c][:, :, None], wv[c][:, :, None], axis=2)
        m_pc[c] = m.transpose(1, 0, 2).reshape(P, NBLK * P).astype(F8)

    # node feature table, padded/bf16
    xt = np.zeros((TOT, F), BF)
    xt[row] = np.asarray(x, np.float32).astype(BF)

    # static schedule: per block -> (window, first/last-of-window);
    # gather calls: chunks of <= CALLBLK blocks within one (g, q) run.
    blk_win = np.zeros(NBLK, np.int64)
    first_blk = {}
    last_blk = {}
    for (g, q, w, b0, nb) in seg_order:
        blk_win[b0:b0 + nb] = w
        if w not in first_blk:
            first_blk[w] = b0
        last_blk[w] = b0 + nb - 1
    blk_first = np.zeros(NBLK, bool)
    blk_last = np.zeros(NBLK, bool)
    for w, b in first_blk.items():
        blk_first[b] = True
    for w, b in last_blk.items():
        blk_last[b] = True

    calls = []   # (b0, nb, quarter, group)
    run_key = None
    run_blocks = []
    runs = []
    for (g, q, w, b0, nb) in seg_order:
        if (g, q) != run_key:
            if run_blocks:
                runs.append((run_key, run_blocks))
            run_key = (g, q)
            run_blocks = []
        run_blocks.append((b0, nb))
    if run_blocks:
        runs.append((run_key, run_blocks))
    for (g, q), blocks in runs:
        b0 = blocks[0][0]
        bend = blocks[-1][0] + blocks[-1][1]
        b = b0
        while b < bend:
            nb = min(CALLBLK, bend - b)
            calls.append((b, nb, q, g))
            b += nb

    # pooling metadata: host-built one-hot graph-slot matrices
    # [128, NW*128]: partition p holds, for window w, onehot(graph-slot of
    # node w*128+p).
    poolm_pc = np.zeros((C, P, NW * P), F8)
    for c in range(C):
        batloc = np.full(NODE_PAD, -1, np.int64)
        nn = int(nk[c])
        batloc[:nn] = batch[node_start[c]:node_start[c + 1]] - c * GPC
        pm = np.zeros((NW, P, P), np.float32)
        valid = batloc >= 0
        bl2 = batloc.reshape(NW, P)
        v2 = valid.reshape(NW, P)
        for w in range(NW):
            pm[w, v2[w], bl2[w][v2[w]]] = 1.0
        poolm_pc[c] = pm.transpose(1, 0, 2).reshape(P, NW * P).astype(F8)

    # self-loop diagonal blocks [128, NW*128] fp8: selfm[p, w*128+p] =
    # 1/deg of node (c, w*128+p); zero on padding rows.
    selfm_pc = np.zeros((C, P, NW * P), F8)
    inv_deg = (1.0 / deg).astype(np.float32)
    for c in range(C):
        v = np.zeros(NODE_PAD, np.float32)
        nn = int(nk[c])
        v[:nn] = inv_deg[node_start[c]:node_start[c + 1]]
        sm = np.zeros((NW, P, P), np.float32)
        sm[:, np.arange(P), np.arange(P)] = v.reshape(NW, P)
        selfm_pc[c] = sm.transpose(1, 0, 2).reshape(P, NW * P).astype(F8)

    gcnt = np.bincount(batch, minlength=G).astype(np.float32)
    counts_pc = np.ones((C, P, 1), np.float32)
    counts_pc[:, :GPC, 0] = gcnt.reshape(C, GPC)

    return dict(
        NODE_PAD=NODE_PAD, NW=NW, TOT=TOT, NBLK=NBLK,
        blk_win=blk_win, blk_first=blk_first, blk_last=blk_last, calls=calls,
        idx_pc=idx_pc, m_pc=m_pc, xt=xt,
        xself_pc=np.ascontiguousarray(
            xt.reshape(C, NODE_PAD, F)),
        selfm_pc=selfm_pc,
        poolm_pc=poolm_pc, counts_pc=counts_pc,
    )


def _build_nc(pre, use_bias=True):
    NW = pre["NW"]
    NBLK = pre["NBLK"]
    TOT = pre["TOT"]
    NODE_PAD = pre["NODE_PAD"]
    blk_win = pre["blk_win"]
    blk_first = pre["blk_first"]
    blk_last = pre["blk_last"]
    calls = pre["calls"]

    _nq = int(os.environ.get("KERNEL_NQUEUES", "4"))
    nc = bacc.Bacc(None, num_devices=C, num_swdge_queues=_nq)

    xt_d = nc.dram_tensor("xt", [TOT, F], bf16, kind="ExternalInput")
    idx_d = nc.dram_tensor("eidx", [128, NBLK * 8], i16, kind="ExternalInput")
    m_d = nc.dram_tensor("emat", [P, NBLK * P], fp8, kind="ExternalInput")
    ones_d = nc.dram_tensor("ones", [1, P], bf16, kind="ExternalInput")
    w1_d = nc.dram_tensor("w1", [F, F], bf16, kind="ExternalInput")
    w2_d = nc.dram_tensor("w2", [F, F], bf16, kind="ExternalInput")
    b1_d = nc.dram_tensor("b1", [1, F], bf16, kind="ExternalInput")
    b2_d = nc.dram_tensor("b2", [1, F], bf16, kind="ExternalInput")
    poolm_d = nc.dram_tensor("poolm", [P, NW * P], fp8, kind="ExternalInput")
    selfm_d = nc.dram_tensor("selfm", [P, NW * P], fp8, kind="ExternalInput")
    xself_d = nc.dram_tensor("xself", [NODE_PAD, F], bf16, kind="ExternalInput")
    cnts_d = nc.dram_tensor("cnts", [P, 1], f32, kind="ExternalInput")
    out_d = nc.dram_tensor("out", [GPC, F], f32, kind="ExternalOutput")

    _ablate = os.environ.get("KERNEL_ABLATE", "full")
    _gbufs = int(os.environ.get("KERNEL_GBUFS", "6"))
    _mbufs = int(os.environ.get("KERNEL_MBUFS", "6"))
    _zbufs = int(os.environ.get("KERNEL_ZBUFS", "2"))
    _pshbufs = int(os.environ.get("KERNEL_PSHBUFS", "2"))
    with tile.TileContext(nc) as tc:
        with (
            tc.tile_pool(name="const", bufs=1) as cpool,
            tc.tile_pool(name="gbuf", bufs=_gbufs) as gpool,
            tc.tile_pool(name="mbuf", bufs=_mbufs) as mpool,
            tc.tile_pool(name="zt", bufs=_zbufs) as ztpool,
            tc.tile_pool(name="hsb", bufs=_zbufs) as hpool,
            tc.tile_pool(name="slb", bufs=4) as slpool,
            tc.tile_pool(name="osb", bufs=2) as opool,
            tc.tile_pool(name="psw", bufs=WG + 1, space="PSUM") as pswpool,
            tc.tile_pool(name="psh", bufs=_pshbufs, space="PSUM") as pshpool,
            tc.tile_pool(name="psp", bufs=1, space="PSUM") as psppool,
            tc.tile_pool(name="dram", bufs=1, space="DRAM") as dpool,
        ):
            # --- constants ---
            idx_sb = cpool.tile([128, NBLK * 8], i16)
            nc.sync.dma_start(out=idx_sb[:], in_=idx_d[:])
            ones_sb = cpool.tile([1, P], bf16)
            nc.sync.dma_start(out=ones_sb[:], in_=ones_d[:])
            w1_sb = cpool.tile([F, F], bf16)
            nc.sync.dma_start(out=w1_sb[:], in_=w1_d[:])
            w2_sb = cpool.tile([F, F], bf16)
            nc.sync.dma_start(out=w2_sb[:], in_=w2_d[:])
            b1_sb = cpool.tile([1, F], bf16)
            nc.sync.dma_start(out=b1_sb[:], in_=b1_d[:])
            b2_sb = cpool.tile([1, F], bf16)
            nc.sync.dma_start(out=b2_sb[:], in_=b2_d[:])
            poolm_sb = cpool.tile([P, NW * P], fp8)
            nc.sync.dma_start(out=poolm_sb[:], in_=poolm_d[:])
            selfm_sb = cpool.tile([P, NW * P], fp8)
            nc.sync.dma_start(out=selfm_sb[:], in_=selfm_d[:])
            cnts_sb = cpool.tile([P, 1], f32)
            nc.sync.dma_start(out=cnts_sb[:], in_=cnts_d[:])

            # Funnel const-tile deps through the Vector engine (the ISA has a
            # small per-instruction sync-wait budget; same-engine ordering is
            # free).
            scratch = cpool.tile([P, 1], f32)
            for t in (w1_sb, w2_sb, poolm_sb, selfm_sb, cnts_sb):
                nc.vector.reduce_sum(out=scratch[:], in_=t[:],
                                     axis=mybir.AxisListType.X)
            for t in (ones_sb, b1_sb, b2_sb):
                nc.vector.reduce_sum(out=scratch[:1, :], in_=t[:],
                                     axis=mybir.AxisListType.X)

            # initialize gather buffers: lanes skipped via negative indices
            # leave SBUF unwritten, and uninitialized bits could be NaN
            # (NaN * 0 would poison PSUM)
            for _ in range(_gbufs):
                gz = gpool.tile([P, CALLBLK, P], bf16, tag="g")
                nc.gpsimd.memset(gz[:], 0)

            h1_shard = dpool.tile([NODE_PAD, F], bf16)
            h1_table = dpool.tile([TOT, F], bf16, addr_space="Shared")
            h1_local = dpool.tile([TOT, F], bf16)
            _skip_l2 = os.environ.get("KERNEL_SKIP_L2", "0") == "1"

            pool_ps = psppool.tile([P, F], f32)

            for layer in range(1 if _skip_l2 else 2):
                _l2_xt = os.environ.get("KERNEL_L2_FROM_XT", "0") == "1"
                table = xt_d if (layer == 0 or _l2_xt) else h1_local
                wmat_sb = w1_sb if layer == 0 else w2_sb
                b_sb = b1_sb if layer == 0 else b2_sb

                ps_tiles = {}
                cur_group = -1
                for ci, (b0, nbk, q, grp) in enumerate(calls):
                    if grp != cur_group and _ablate not in ("gather", "gm"):
                        cur_group = grp
                        for w in range(grp * WG, min((grp + 1) * WG, NW)):
                            ps_tiles[w] = pswpool.tile(
                                [P, P], f32, tag="psw", name=f"psw{w % 8}")
                            sl_t = slpool.tile([P, F], bf16, tag="sl")
                            src_tab = xself_d if layer == 0 else h1_shard
                            nc.sync.dma_start(
                                out=sl_t[:],
                                in_=src_tab[w * P:(w + 1) * P, :],
                            )
                            nc.tensor.matmul(
                                ps_tiles[w][:],
                                lhsT=sl_t[:],
                                rhs=selfm_sb[:, w * P:(w + 1) * P],
                                start=True, stop=False,
                            )
                    g_t = gpool.tile([P, CALLBLK, P], bf16, tag="g")
                    nc.gpsimd.dma_gather(
                        out_ap=g_t[:, :nbk, :],
                        in_ap=table[q * QROWS:min((q + 1) * QROWS, TOT), :],
                        idxs_ap=idx_sb[:, b0 * 8:(b0 + nbk) * 8],
                        num_idxs=nbk * P,
                        num_idxs_reg=nbk * P,
                        elem_size=F,
                        queue_num=ci % _nq,
                        single_packet=os.environ.get("KERNEL_SP", "1") == "1",
                    )
                    if _ablate == "gather":
                        continue
                    m_t = mpool.tile([P, CALLBLK * P], fp8, tag="m")
                    nc.sync.dma_start(
                        out=m_t[:, :nbk * P],
                        in_=m_d[:, b0 * P:(b0 + nbk) * P],
                    )
                    if _ablate == "gm":
                        continue
                    for j in range(nbk):
                        blk = b0 + j
                        w = int(blk_win[blk])
                        ps_w = ps_tiles[w]
                        nc.tensor.matmul(
                            ps_w[:],
                            lhsT=g_t[:, j, :],
                            rhs=m_t[:, j * P:(j + 1) * P],
                            start=False,
                            stop=bool(blk_last[blk]),
                        )
                        if blk_last[blk]:
                            # ---- dense part for finished window w ----
                            zt = ztpool.tile([P, P], bf16, tag="zt")
                            nc.vector.tensor_copy(out=zt[:], in_=ps_w[:])
                            del ps_tiles[w]
                            if _ablate == "agg":
                                continue
                            ps_h = pshpool.tile([P, F], f32, tag="psh")
                            nc.tensor.matmul(
                                ps_h[:], lhsT=zt[:], rhs=wmat_sb[:],
                                start=True, stop=not use_bias,
                            )
                            if use_bias:
                                nc.tensor.matmul(
                                    ps_h[:], lhsT=ones_sb[:], rhs=b_sb[:],
                                    start=False, stop=True,
                                )
                            h_sb = hpool.tile([P, F], bf16, tag="h")
                            nc.scalar.activation(
                                out=h_sb[:], in_=ps_h[:],
                                func=mybir.ActivationFunctionType.Relu,
                            )
                            if layer == 0:
                                nc.sync.dma_start(
                                    out=h1_shard[w * P:(w + 1) * P, :],
                                    in_=h_sb[:],
                                )
                            else:
                                nc.tensor.matmul(
                                    pool_ps[:],
                                    lhsT=poolm_sb[:, w * P:(w + 1) * P],
                                    rhs=h_sb[:],
                                    start=(w == 0),
                                    stop=(w == NW - 1),
                                )

                if layer == 0 and not _skip_l2:
                    _ag_local = os.environ.get("KERNEL_AG_LOCAL", "1") == "1"
                    if _ag_local:
                        nc.gpsimd.collective_compute(
                            "AllGather",
                            mybir.AluOpType.bypass,
                            replica_groups=[list(range(C))],
                            ins=[h1_shard[:]],
                            outs=[h1_local[:]],
                        )
                    else:
                        nc.gpsimd.collective_compute(
                            "AllGather",
                            mybir.AluOpType.bypass,
                            replica_groups=[list(range(C))],
                            ins=[h1_shard[:]],
                            outs=[h1_table[:]],
                        )
                        # dma_gather from Shared-space DRAM is slower than
                        # from Local; stage the table locally first.
                        nc.sync.dma_start(out=h1_local[:], in_=h1_table[:])

            # ---- finalize pool: divide by counts ----
            if _skip_l2 or _ablate != "full":
                # touch pool_ps so it exists; output is meaningless
                nc.tensor.matmul(pool_ps[:], lhsT=ones_sb[:], rhs=b1_sb[:],
                                 start=True, stop=True)
            rec_sb = opool.tile([P, 1], f32, tag="rec")
            nc.vector.reciprocal(out=rec_sb[:], in_=cnts_sb[:])
            out_sb = opool.tile([P, F], f32, tag="os")
            nc.vector.tensor_scalar(
                out=out_sb[:],
                in0=pool_ps[:],
                scalar1=rec_sb[:, 0:1],
                scalar2=None,
                op0=mybir.AluOpType.mult,
            )
            nc.sync.dma_start(out=out_d[:], in_=out_sb[0:GPC, :])

    nc.compile()
    return nc


def kernel(x, edge_index, batch, W1, b1, W2, b2):
    x = np.asarray(x, np.float32)
    pre = _preprocess(x, edge_index, batch)

    ones = np.ones((1, P), BF)
    w1b = np.asarray(W1, np.float32).astype(BF)
    w2b = np.asarray(W2, np.float32).astype(BF)
    b1b = np.asarray(b1, np.float32).reshape(1, F).astype(BF)
    b2b = np.asarray(b2, np.float32).reshape(1, F).astype(BF)

    in_maps = []
    for c in range(C):
        in_maps.append({
            "xt": pre["xt"],
            "eidx": pre["idx_pc"][c],
            "emat": pre["m_pc"][c],
            "ones": ones,
            "w1": w1b,
            "w2": w2b,
            "b1": b1b,
            "b2": b2b,
            "poolm": pre["poolm_pc"][c],
            "selfm": pre["selfm_pc"][c],
            "xself": pre["xself_pc"][c],
            "cnts": pre["counts_pc"][c],
        })

    use_bias = bool(np.any(b1b.astype(np.float32))) or bool(np.any(b2b.astype(np.float32)))
    nc = _build_nc(pre, use_bias=use_bias)
    res = run_bass_kernel_spmd(nc, in_maps, core_ids=list(range(C)))
    out = np.concatenate([res.results[c]["out"] for c in range(C)], axis=0)
    return out.astype(np.float32)


# revision 21
# speedup vs baseline: 1.6834x; 1.6834x over previous
"""Trainium2 Bass kernel for a 2-layer GCN encoder + global mean pool.

Problem: x[100000,128] f32, edge_index[2,1600000] i64, batch[100000] i64
(sorted), W1/b1/W2/b2. Two GCNConv layers (symmetric deg^-1/2 norm, self
loops, relu) then mean-pool over 512 graphs -> [512, 128] f32.

Strategy (8 NeuronCores, data-parallel over graphs):
- Nodes partitioned by graph id into 8 contiguous shards (batch is sorted);
  each core owns the edges whose *destination* lands in its shard.
- Algebraic rewrite: aggregate-then-transform.  For each layer,
      h' = relu( (A_hat @ h) @ W + b ),   A_hat = D^-1/2 (A+I) D^-1/2
  so the sparse aggregation runs on raw 128-dim features and the dense
  matmul with W happens per 128-node window afterwards.  Self loops are
  NOT gathered: each window's own 128 rows are contiguous in the table,
  so they are streamed with a plain 2-descriptor DMA (from the per-core
  shard copy in layer 1, from the local h1 shard in layer 2 — no
  AllGather dependency) and folded in through a diagonal 1/deg block
  that opens each window's PSUM accumulation.
- Sparse aggregation per core: per 128-edge block, dma_gather fetches the
  128 source rows (bf16, 256B each) from the feature table; a host-built
  fp8-e4m3 selection matrix M_blk (one nonzero per edge row: its dst slot,
  valued with the GCN edge weight) is streamed from HBM as a plain
  contiguous DMA and scatters the gathered rows on the Tensor engine:
      PSUM[f, s] += G_blk.T @ M_blk
  accumulated over all blocks of a 128-node destination window.  The M
  matrices are identical for both layers (same graph), so one table in
  HBM serves both.  Padded gather lanes carry spread-out indices (a row-0
  default creates an HBM hotspot worth ~50% of gather time).
- dma_gather uses int16 indices, so the node table is addressed through
  <=32768-row "quarter" slices; edges are sorted by (window-group, quarter,
  window) so each gather call stays quarter-pure while the PSUM tiles of a
  4-window group stay resident across the group's quarter runs.
- The finished PSUM window (= Z^T) feeds the dense W matmul directly (its
  transposed layout is exactly the lhsT the Tensor engine wants); bias is
  added via a K=1 matmul with a ones row; relu on the Scalar engine.
- Between layers one 8-core AllGather shares the per-shard h1 table (bf16),
  written directly into local DRAM (gathering from Shared-space DRAM, or
  staging it through a copy, costs ~0.5 ms extra).
- Mean pool: host-built one-hot graph-slot matrices (streamed once) feed
  per-window matmuls into a single resident PSUM bank, then a
  reciprocal-count scale.

Host-side preprocessing is purely structural (index sorting, degree counts,
normalization coefficients and one-hot selection matrices derived from the
graph topology); all feature/weight compute runs on device.
"""

import math
import os

import numpy as np
import ml_dtypes

import concourse.bass as bass
import concourse.bacc as bacc
import concourse.mybir as mybir
import concourse.tile as tile
from concourse.bass_utils import run_bass_kernel_spmd

P = 128
C = 8               # cores
G = 512             # graphs
GPC = G // C        # graphs per core
F = 128             # feature dim (in = hid = out)
WG = 4              # windows per PSUM-resident group
QROWS = 32768       # rows per int16-addressable table slice

bf16 = mybir.dt.bfloat16
fp8 = mybir.dt.float8e4
f32 = mybir.dt.float32
i16 = mybir.dt.int16

BF = ml_dtypes.bfloat16
F8 = ml_dtypes.float8_e4m3

CALLBLK = int(os.environ.get("KERNEL_CALLBLK", "8"))


def _preprocess(x, edge_index, batch):
    """Structural preprocessing: shard nodes by graph, sort/pad edges by
    (window-group, src-quarter, dst-window), compute GCN norm weights and
    host-built selection matrices."""
    N = x.shape[0]
    src = np.asarray(edge_index[0], dtype=np.int64)
    dst = np.asarray(edge_index[1], dtype=np.int64)
    batch = np.asarray(batch, dtype=np.int64)

    node_start = np.searchsorted(batch, np.arange(C + 1) * GPC).astype(np.int64)
    nk = np.diff(node_start)
    NODE_PAD = int(math.ceil(nk.max() / P) * P)
    NW = NODE_PAD // P
    TOT = C * NODE_PAD
    assert TOT <= 4 * QROWS
    NG = (NW + WG - 1) // WG

    core_of = (batch // GPC).astype(np.int64)
    row = (np.arange(N) - node_start[core_of] + core_of * NODE_PAD).astype(np.int64)

    deg = np.bincount(dst, minlength=N).astype(np.float64) + 1.0
    dis = 1.0 / np.sqrt(deg)

    # real edges only; self loops are handled by streaming each window's
    # own (contiguous) rows through a diagonal 1/deg block instead of
    # spending 128 random-gather descriptors per window on them.
    esrc = src
    edst = dst
    ew = (dis[src] * dis[dst]).astype(np.float32)

    ecore = core_of[edst]
    eld = edst - node_start[ecore]
    ewin = eld >> 7
    eslot = (eld & 127).astype(np.int64)
    esrcrow = row[esrc]
    eq = (esrcrow // QROWS).astype(np.int64)      # source quarter
    eloc = (esrcrow % QROWS).astype(np.int64)     # quarter-local row
    egrp = ewin // WG

    NQ = 4
    # segment id in (core, group, quarter, window) order
    seg = ((ecore * NG + egrp) * NQ + eq) * NW + ewin
    NSEG = C * NG * NQ * NW
    counts = np.bincount(seg, minlength=NSEG)
    cnt4 = counts.reshape(C, NG, NQ, NW)
    # SPMD-common block counts per (group, quarter, window)
    BWS = np.ceil(cnt4.max(axis=0) / P).astype(np.int64)      # [NG, NQ, NW]
    for g in range(NG):
        mask = np.zeros(NW, bool)
        mask[g * WG:(g + 1) * WG] = True
        BWS[g, :, ~mask] = 0
    # every window needs >= 1 block so its PSUM tile is always written
    for w in range(NW):
        g = w // WG
        if BWS[g, :, w].sum() == 0:
            BWS[g, 0, w] = 1
    NBLK = int(BWS.sum())

    # block/segment offsets in (g, q, w) order
    seg_order = []          # (g, q, w, block_start, nblocks)
    seg_start = np.zeros((NG, NQ, NW), np.int64)
    acc = 0
    for g in range(NG):
        for q in range(NQ):
            for w in range(g * WG, min((g + 1) * WG, NW)):
                seg_start[g, q, w] = acc
                nb = int(BWS[g, q, w])
                if nb:
                    seg_order.append((g, q, w, acc, nb))
                acc += nb
    assert acc == NBLK

    # scatter edges into the padded per-core layout
    order = np.argsort(seg, kind="stable")
    seg_sorted = seg[order]
    grp_excl = np.concatenate([[0], np.cumsum(counts)[:-1]])
    pos = np.arange(order.size) - grp_excl[seg_sorted]
    es = order
    dest = (ecore[es] * (NBLK * P)
            + seg_start[egrp[es], eq[es], ewin[es]] * P + pos)

    # Padded lanes carry zero weight in M, so their index value is
    # correctness-neutral — spread them uniformly over the quarter instead
    # of defaulting to row 0 (duplicate descriptors to one HBM row create a
    # bank hotspot that slows the whole gather by ~50%).
    blk_q = np.zeros(NBLK, np.int64)
    for (g, q, w, b0, nb) in seg_order:
        blk_q[b0:b0 + nb] = q
    qrows_of_blk = np.where(blk_q < (TOT // QROWS), QROWS, TOT - (TOT // QROWS) * QROWS)
    prng = np.random.default_rng(12345)
    spread = prng.integers(0, 1 << 30, size=(C, NBLK, P)) % qrows_of_blk[None, :, None]
    idx_arr = spread.reshape(-1).astype(np.int16)   # quarter-local src row
    slot_arr = np.zeros(C * NBLK * P, np.int64)
    w_arr = np.zeros(C * NBLK * P, np.float32)
    idx_arr[dest] = eloc[es].astype(np.int16)
    slot_arr[dest] = eslot[es]
    w_arr[dest] = ew[es]

    if os.environ.get("KERNEL_SORT_LANES", "0") == "1":
        # sort lanes within each 128-lane block by gather index
        ia = idx_arr.reshape(C, NBLK, P)
        sa = slot_arr.reshape(C, NBLK, P)
        wa = w_arr.reshape(C, NBLK, P)
        o = np.argsort(ia, axis=2, kind="stable")
        idx_arr = np.take_along_axis(ia, o, 2).reshape(-1)
        slot_arr = np.take_along_axis(sa, o, 2).reshape(-1)
        w_arr = np.take_along_axis(wa, o, 2).reshape(-1)

    # per-core uploads
    # idx: wrapped [16, NBLK*8] (logical i at [i%16, i//16]), replicated to
    # 128 partitions (the gather ucode's per-Q7-core channel groups all read
    # the same wrap)
    idx_pc = np.ascontiguousarray(
        idx_arr.reshape(C, NBLK * P // 16, 16).transpose(0, 2, 1))
    idx_pc = np.ascontiguousarray(np.tile(idx_pc, (1, 8, 1)))

    # host-built selection matrices, transposed layout [128, NBLK*128]:
    # partition p holds, for each block b, the row M_b[p, :] (one nonzero
    # at the dst slot of the edge in lane p of block b).
    m_pc = np.zeros((C, P, NBLK * P), F8)
    sl = slot_arr.reshape(C, NBLK, P)
    wv = w_arr.reshape(C, NBLK, P)
    for c in range(C):
        m = np.zeros((NBLK, P, P), np.float32)
        np.put_along_axis(m, sl[c][:, :, None], wv[c][:, :, None], axis=2)
        m_pc[c] = m.transpose(1, 0, 2).reshape(P, NBLK * P).astype(F8)

    # node feature table, padded/bf16
    xt = np.zeros((TOT, F), BF)
    xt[row] = np.asarray(x, np.float32).astype(BF)

    # static schedule: per block -> (window, first/last-of-window);
    # gather calls: chunks of <= CALLBLK blocks within one (g, q) run.
    blk_win = np.zeros(NBLK, np.int64)
    first_blk = {}
    last_blk = {}
    for (g, q, w, b0, nb) in seg_order:
        blk_win[b0:b0 + nb] = w
        if w not in first_blk:
            first_blk[w] = b0
        last_blk[w] = b0 + nb - 1
    blk_first = np.zeros(NBLK, bool)
    blk_last = np.zeros(NBLK, bool)
    for w, b in first_blk.items():
        blk_first[b] = True
    for w, b in last_blk.items():
        blk_last[b] = True

    calls = []   # (b0, nb, quarter, group)
    run_key = None
    run_blocks = []
    runs = []
    for (g, q, w, b0, nb) in seg_order:
        if (g, q) != run_key:
            if run_blocks:
                runs.append((run_key, run_blocks))
            run_key = (g, q)
            run_blocks = []
        run_blocks.append((b0, nb))
    if run_blocks:
        runs.append((run_key, run_blocks))
    for (g, q), blocks in runs:
        b0 = blocks[0][0]
        bend = blocks[-1][0] + blocks[-1][1]
        b = b0
        while b < bend:
            nb = min(CALLBLK, bend - b)
            calls.append((b, nb, q, g))
            b += nb

    # pooling metadata: host-built one-hot graph-slot matrices
    # [128, NW*128]: partition p holds, for window w, onehot(graph-slot of
    # node w*128+p).
    poolm_pc = np.zeros((C, P, NW * P), F8)
    for c in range(C):
        batloc = np.full(NODE_PAD, -1, np.int64)
        nn = int(nk[c])
        batloc[:nn] = batch[node_start[c]:node_start[c + 1]] - c * GPC
        pm = np.zeros((NW, P, P), np.float32)
        valid = batloc >= 0
        bl2 = batloc.reshape(NW, P)
        v2 = valid.reshape(NW, P)
        for w in range(NW):
            pm[w, v2[w], bl2[w][v2[w]]] = 1.0
        poolm_pc[c] = pm.transpose(1, 0, 2).reshape(P, NW * P).astype(F8)

    # self-loop diagonal blocks [128, NW*128] fp8: selfm[p, w*128+p] =
    # 1/deg of node (c, w*128+p); zero on padding rows.
    selfm_pc = np.zeros((C, P, NW * P), F8)
    inv_deg = (1.0 / deg).astype(np.float32)
    for c in range(C):
        v = np.zeros(NODE_PAD, np.float32)
        nn = int(nk[c])
        v[:nn] = inv_deg[node_start[c]:node_start[c + 1]]
        sm = np.zeros((NW, P, P), np.float32)
        sm[:, np.arange(P), np.arange(P)] = v.reshape(NW, P)
        selfm_pc[c] = sm.transpose(1, 0, 2).reshape(P, NW * P).astype(F8)

    gcnt = np.bincount(batch, minlength=G).astype(np.float32)
    counts_pc = np.ones((C, P, 1), np.float32)
    counts_pc[:, :GPC, 0] = gcnt.reshape(C, GPC)

    return dict(
        NODE_PAD=NODE_PAD, NW=NW, TOT=TOT, NBLK=NBLK,
        blk_win=blk_win, blk_first=blk_first, blk_last=blk_last, calls=calls,
        idx_pc=idx_pc, m_pc=m_pc, xt=xt,
        xself_pc=np.ascontiguousarray(
            xt.reshape(C, NODE_PAD, F)),
        selfm_pc=selfm_pc,
        poolm_pc=poolm_pc, counts_pc=counts_pc,
    )


def _build_nc(pre, use_bias=True):
    NW = pre["NW"]
    NBLK = pre["NBLK"]
    TOT = pre["TOT"]
    NODE_PAD = pre["NODE_PAD"]
    blk_win = pre["blk_win"]
    blk_first = pre["blk_first"]
    blk_last = pre["blk_last"]
    calls = pre["calls"]

    _nq = int(os.environ.get("KERNEL_NQUEUES", "4"))
    nc = bacc.Bacc(None, num_devices=C, num_swdge_queues=_nq)

    xt_d = nc.dram_tensor("xt", [TOT, F], bf16, kind="ExternalInput")
    idx_d = nc.dram_tensor("eidx", [128, NBLK * 8], i16, kind="ExternalInput")
    m_d = nc.dram_tensor("emat", [P, NBLK * P], fp8, kind="ExternalInput")
    ones_d = nc.dram_tensor("ones", [1, P], bf16, kind="ExternalInput")
    w1_d = nc.dram_tensor("w1", [F, F], bf16, kind="ExternalInput")
    w2_d = nc.dram_tensor("w2", [F, F], bf16, kind="ExternalInput")
    b1_d = nc.dram_tensor("b1", [1, F], bf16, kind="ExternalInput")
    b2_d = nc.dram_tensor("b2", [1, F], bf16, kind="ExternalInput")
    poolm_d = nc.dram_tensor("poolm", [P, NW * P], fp8, kind="ExternalInput")
    selfm_d = nc.dram_tensor("selfm", [P, NW * P], fp8, kind="ExternalInput")
    xself_d = nc.dram_tensor("xself", [NODE_PAD, F], bf16, kind="ExternalInput")
    cnts_d = nc.dram_tensor("cnts", [P, 1], f32, kind="ExternalInput")
    out_d = nc.dram_tensor("out", [GPC, F], f32, kind="ExternalOutput")

    _ablate = os.environ.get("KERNEL_ABLATE", "full")
    _gbufs = int(os.environ.get("KERNEL_GBUFS", "6"))
    _mbufs = int(os.environ.get("KERNEL_MBUFS", "6"))
    _zbufs = int(os.environ.get("KERNEL_ZBUFS", "2"))
    _pshbufs = int(os.environ.get("KERNEL_PSHBUFS", "2"))
    with tile.TileContext(nc) as tc:
        with (
            tc.tile_pool(name="const", bufs=1) as cpool,
            tc.tile_pool(name="gbuf", bufs=_gbufs) as gpool,
            tc.tile_pool(name="mbuf", bufs=_mbufs) as mpool,
            tc.tile_pool(name="zt", bufs=_zbufs) as ztpool,
            tc.tile_pool(name="hsb", bufs=_zbufs) as hpool,
            tc.tile_pool(name="slb", bufs=4) as slpool,
            tc.tile_pool(name="osb", bufs=2) as opool,
            tc.tile_pool(name="psw", bufs=WG + 1, space="PSUM") as pswpool,
            tc.tile_pool(name="psh", bufs=_pshbufs, space="PSUM") as pshpool,
            tc.tile_pool(name="psp", bufs=1, space="PSUM") as psppool,
            tc.tile_pool(name="dram", bufs=1, space="DRAM") as dpool,
        ):
            # --- constants ---
            idx_sb = cpool.tile([128, NBLK * 8], i16)
            nc.sync.dma_start(out=idx_sb[:], in_=idx_d[:])
            ones_sb = cpool.tile([1, P], bf16)
            nc.sync.dma_start(out=ones_sb[:], in_=ones_d[:])
            w1_sb = cpool.tile([F, F], bf16)
            nc.sync.dma_start(out=w1_sb[:], in_=w1_d[:])
            w2_sb = cpool.tile([F, F], bf16)
            nc.sync.dma_start(out=w2_sb[:], in_=w2_d[:])
            b1_sb = cpool.tile([1, F], bf16)
            nc.sync.dma_start(out=b1_sb[:], in_=b1_d[:])
            b2_sb = cpool.tile([1, F], bf16)
            nc.sync.dma_start(out=b2_sb[:], in_=b2_d[:])
            poolm_sb = cpool.tile([P, NW * P], fp8)
            nc.sync.dma_start(out=poolm_sb[:], in_=poolm_d[:])
            selfm_sb = cpool.tile([P, NW * P], fp8)
            nc.sync.dma_start(out=selfm_sb[:], in_=selfm_d[:])
            cnts_sb = cpool.tile([P, 1], f32)
            nc.sync.dma_start(out=cnts_sb[:], in_=cnts_d[:])

            # Funnel const-tile deps through the Vector engine (the ISA has a
            # small per-instruction sync-wait budget; same-engine ordering is
            # free).
            scratch = cpool.tile([P, 1], f32)
            for t in (w1_sb, w2_sb, poolm_sb, selfm_sb, cnts_sb):
                nc.vector.reduce_sum(out=scratch[:], in_=t[:],
                                     axis=mybir.AxisListType.X)
            for t in (ones_sb, b1_sb, b2_sb):
                nc.vector.reduce_sum(out=scratch[:1, :], in_=t[:],
                                     axis=mybir.AxisListType.X)

            h1_shard = dpool.tile([NODE_PAD, F], bf16)
            h1_table = dpool.tile([TOT, F], bf16, addr_space="Shared")
            h1_local = dpool.tile([TOT, F], bf16)
            _skip_l2 = os.environ.get("KERNEL_SKIP_L2", "0") == "1"

            pool_ps = psppool.tile([P, F], f32)

            for layer in range(1 if _skip_l2 else 2):
                _l2_xt = os.environ.get("KERNEL_L2_FROM_XT", "0") == "1"
                table = xt_d if (layer == 0 or _l2_xt) else h1_local
                wmat_sb = w1_sb if layer == 0 else w2_sb
                b_sb = b1_sb if layer == 0 else b2_sb

                ps_tiles = {}
                cur_group = -1
                for ci, (b0, nbk, q, grp) in enumerate(calls):
                    if grp != cur_group and _ablate not in ("gather", "gm"):
                        cur_group = grp
                        for w in range(grp * WG, min((grp + 1) * WG, NW)):
                            ps_tiles[w] = pswpool.tile(
                                [P, P], f32, tag="psw", name=f"psw{w % 8}")
                            sl_t = slpool.tile([P, F], bf16, tag="sl")
                            src_tab = xself_d if layer == 0 else h1_shard
                            nc.sync.dma_start(
                                out=sl_t[:],
                                in_=src_tab[w * P:(w + 1) * P, :],
                            )
                            nc.tensor.matmul(
                                ps_tiles[w][:],
                                lhsT=sl_t[:],
                                rhs=selfm_sb[:, w * P:(w + 1) * P],
                                start=True, stop=False,
                            )
                    g_t = gpool.tile([P, CALLBLK, P], bf16, tag="g")
                    nc.gpsimd.dma_gather(
                        out_ap=g_t[:, :nbk, :],
                        in_ap=table[q * QROWS:min((q + 1) * QROWS, TOT), :],
                        idxs_ap=idx_sb[:, b0 * 8:(b0 + nbk) * 8],
                        num_idxs=nbk * P,
                        num_idxs_reg=nbk * P,
                        elem_size=F,
                        queue_num=ci % _nq,
                        single_packet=os.environ.get("KERNEL_SP", "1") == "1",
                    )
                    if _ablate == "gather":
                        continue
                    m_t = mpool.tile([P, CALLBLK * P], fp8, tag="m")
                    nc.sync.dma_start(
                        out=m_t[:, :nbk * P],
                        in_=m_d[:, b0 * P:(b0 + nbk) * P],
                    )
                    if _ablate == "gm":
                        continue
                    for j in range(nbk):
                        blk = b0 + j
                        w = int(blk_win[blk])
                        ps_w = ps_tiles[w]
                        nc.tensor.matmul(
                            ps_w[:],
                            lhsT=g_t[:, j, :],
                            rhs=m_t[:, j * P:(j + 1) * P],
                            start=False,
                            stop=bool(blk_last[blk]),
                        )
                        if blk_last[blk]:
                            # ---- dense part for finished window w ----
                            zt = ztpool.tile([P, P], bf16, tag="zt")
                            nc.vector.tensor_copy(out=zt[:], in_=ps_w[:])
                            del ps_tiles[w]
                            if _ablate == "agg":
                                continue
                            ps_h = pshpool.tile([P, F], f32, tag="psh")
                            nc.tensor.matmul(
                                ps_h[:], lhsT=zt[:], rhs=wmat_sb[:],
                                start=True, stop=not use_bias,
                            )
                            if use_bias:
                                nc.tensor.matmul(
                                    ps_h[:], lhsT=ones_sb[:], rhs=b_sb[:],
                                    start=False, stop=True,
                                )
                            h_sb = hpool.tile([P, F], bf16, tag="h")
                            nc.scalar.activation(
                                out=h_sb[:], in_=ps_h[:],
                                func=mybir.ActivationFunctionType.Relu,
                            )
                            if layer == 0:
                                nc.sync.dma_start(
                                    out=h1_shard[w * P:(w + 1) * P, :],
                                    in_=h_sb[:],
                                )
                            else:
                                nc.tensor.matmul(
                                    pool_ps[:],
                                    lhsT=poolm_sb[:, w * P:(w + 1) * P],
                                    rhs=h_sb[:],
                                    start=(w == 0),
                                    stop=(w == NW - 1),
                                )

                if layer == 0 and not _skip_l2:
                    _ag_local = os.environ.get("KERNEL_AG_LOCAL", "1") == "1"
                    if _ag_local:
                        nc.gpsimd.collective_compute(
                            "AllGather",
                            mybir.AluOpType.bypass,
                            replica_groups=[list(range(C))],
                            ins=[h1_shard[:]],
                            outs=[h1_local[:]],
                        )
                    else:
                        nc.gpsimd.collective_compute(
                            "AllGather",
                            mybir.AluOpType.bypass,
                            replica_groups=[list(range(C))],
                            ins=[h1_shard[:]],
                            outs=[h1_table[:]],
                        )
                        # dma_gather from Shared-space DRAM is slower than
                        # from Local; stage the table locally first.
                        nc.sync.dma_start(out=h1_local[:], in_=h1_table[:])

            # ---- finalize pool: divide by counts ----
            if _skip_l2 or _ablate != "full":
                # touch pool_ps so it exists; output is meaningless
                nc.tensor.matmul(pool_ps[:], lhsT=ones_sb[:], rhs=b1_sb[:],
                                 start=True, stop=True)
            rec_sb = opool.tile([P, 1], f32, tag="rec")
            nc.vector.reciprocal(out=rec_sb[:], in_=cnts_sb[:])
            out_sb = opool.tile([P, F], f32, tag="os")
            nc.vector.tensor_scalar(
                out=out_sb[:],
                in0=pool_ps[:],
                scalar1=rec_sb[:, 0:1],
                scalar2=None,
                op0=mybir.AluOpType.mult,
            )
            nc.sync.dma_start(out=out_d[:], in_=out_sb[0:GPC, :])

    nc.compile()
    return nc


def kernel(x, edge_index, batch, W1, b1, W2, b2):
    x = np.asarray(x, np.float32)
    pre = _preprocess(x, edge_index, batch)

    ones = np.ones((1, P), BF)
    w1b = np.asarray(W1, np.float32).astype(BF)
    w2b = np.asarray(W2, np.float32).astype(BF)
    b1b = np.asarray(b1, np.float32).reshape(1, F).astype(BF)
    b2b = np.asarray(b2, np.float32).reshape(1, F).astype(BF)

    in_maps = []
    for c in range(C):
        in_maps.append({
            "xt": pre["xt"],
            "eidx": pre["idx_pc"][c],
            "emat": pre["m_pc"][c],
            "ones": ones,
            "w1": w1b,
            "w2": w2b,
            "b1": b1b,
            "b2": b2b,
            "poolm": pre["poolm_pc"][c],
            "selfm": pre["selfm_pc"][c],
            "xself": pre["xself_pc"][c],
            "cnts": pre["counts_pc"][c],
        })

    use_bias = bool(np.any(b1b.astype(np.float32))) or bool(np.any(b2b.astype(np.float32)))
    nc = _build_nc(pre, use_bias=use_bias)
    res = run_bass_kernel_spmd(nc, in_maps, core_ids=list(range(C)))
    out = np.concatenate([res.results[c]["out"] for c in range(C)], axis=0)
    return out.astype(np.float32)
